# revision 19
# baseline (speedup 1.0000x reference)
"""Trainium2 Bass kernel: 16-head RoPE attention (B=2, L=2048, HIDDEN=1024).

Sharding: 8 cores = 2 batches x 4 head-groups (4 heads per core).
Attention storage (rope'd q/k, v, exp output) is bf16: score/PV matmuls
issue ~24% faster than fp32r on TRN2 and halve the SBUF traffic, at
rel err ~4e-3 (vs 5e-4 all-fp32r), well inside the 2e-2 gate.
Each core computes q/k/v projections for its 4 heads (feature-major),
RoPE, scores-transposed [k,q] per head, exp (no max subtraction --
scores are ~N(0,1)), PV with a ones-column in V to get softmax sums,
normalization, and a partial output projection [1024, 2048].
Host sums the 4 partials per batch and transposes back.
"""

import numpy as np
import ml_dtypes
from contextlib import ExitStack

from concourse import bacc, tile, mybir
from concourse.bass import ts
from concourse.bass_utils import run_bass_kernel_spmd

HIDDEN = 1024
HEADS = 16
HD = 64
L = 2048
B = 2
BASE = 10000.0

P = 128
E_LOCAL = 256          # 4 heads per core
N_PAIRS = 2            # head pairs per core (2 heads on 128 partitions)
HC = HIDDEN // P       # 8 hidden chunks
TC = 512               # token chunk (matmul free dim)
N_TC = L // TC         # 4
N_TT = L // P          # 16 token tiles (for v / k-tiles)
SCALE = 1.0 / 8.0      # 1/sqrt(HD)

F32 = mybir.dt.float32
F32R = mybir.dt.float32r
BF16 = mybir.dt.bfloat16
AF = mybir.ActivationFunctionType
ALU = mybir.AluOpType


def r(ap):
    """View an fp32 AP as float32r for full-rate PE matmuls."""
    return ap.bitcast(F32R)


def build_program(debug=False):
    nc = bacc.Bacc(None, target_bir_lowering=False)
    names = {}
    with tile.TileContext(nc) as tc:
        ctx = ExitStack()
        with ctx:
            dram = ctx.enter_context(tc.tile_pool(name="dram", bufs=1, space="DRAM"))
            xT_d = dram.tile([HIDDEN, L], BF16, kind="ExternalInput", name="xT")
            wq_d = dram.tile([HIDDEN, E_LOCAL], BF16, kind="ExternalInput", name="wq")
            wk_d = dram.tile([HIDDEN, E_LOCAL], BF16, kind="ExternalInput", name="wk")
            wv_d = dram.tile([HIDDEN, E_LOCAL], BF16, kind="ExternalInput", name="wv")
            wo_d = dram.tile([E_LOCAL, HIDDEN], BF16, kind="ExternalInput", name="wo")
            cos_d = dram.tile([P, L], F32, kind="ExternalInput", name="cosT")
            sin_d = dram.tile([P, L], F32, kind="ExternalInput", name="sinT")
            out_d = dram.tile([HIDDEN, L], BF16, kind="ExternalOutput", name="outT")
            if debug:
                dbg_q = dram.tile([P, L], F32, kind="ExternalOutput", name="dbg_q")
                dbg_k = dram.tile([P, L], F32, kind="ExternalOutput", name="dbg_k")
                dbg_v = dram.tile([P, N_TT * 4 * (HD + 1)], F32, kind="ExternalOutput", name="dbg_v")
                dbg_o = dram.tile([P, L], F32, kind="ExternalOutput", name="dbg_o")
                dbg_ot = dram.tile([HD + 1, 2 * TC], F32, kind="ExternalOutput", name="dbg_ot")
                dbg_inv = dram.tile([1, 2 * TC], F32, kind="ExternalOutput", name="dbg_inv")
                dbg_bsum = dram.tile([HD, 2 * TC], F32, kind="ExternalOutput", name="dbg_bsum")
                names["dbg"] = [t.tensor.name for t in (dbg_q, dbg_k, dbg_v, dbg_o, dbg_ot, dbg_inv, dbg_bsum)]
            names["in"] = ["xT", "wq", "wk", "wv", "wo", "cosT", "sinT"]
            names["out"] = "outT"
            names["in"] = [t.tensor.name for t in (xT_d, wq_d, wk_d, wv_d, wo_d, cos_d, sin_d)]
            names["out"] = out_d.tensor.name

            # ---------------- persistent SBUF ----------------
            const = ctx.enter_context(tc.tile_pool(name="const", bufs=1))
            wq_sb = const.tile([P, HC, E_LOCAL], BF16)
            wk_sb = const.tile([P, HC, E_LOCAL], BF16)
            wv_sb = const.tile([P, HC, E_LOCAL], BF16)
            wo_sb = const.tile([P, 2, HIDDEN], BF16)
            cos_sb = const.tile([P, L], F32)
            sin_sb = const.tile([P, L], F32)
            nc.sync.dma_start(wq_sb[:], wq_d[:].rearrange("(c p) e -> p c e", p=P))
            nc.gpsimd.dma_start(wk_sb[:], wk_d[:].rearrange("(c p) e -> p c e", p=P))

            # rope'd q and k, feature-major: per pair [128, L]
            qkro = ctx.enter_context(tc.tile_pool(name="qkro", bufs=1))
            q_ro = [qkro.tile([P, L], BF16, name=f"q_ro{p}") for p in range(N_PAIRS)]
            k_ro = [qkro.tile([P, L], BF16, name=f"k_ro{p}") for p in range(N_PAIRS)]
            # v token-major with ones columns: [128 tok, tt, 4*65]
            v_all = qkro.tile([P, N_TT, 4 * P], BF16)
            v4 = v_all[:].rearrange("p t (g c) -> p t g c", g=4)
            ones_sb = qkro.tile([P, N_TT], F32)
            nc.vector.memset(ones_sb[:], 1.0)
            for g in range(4):
                nc.vector.tensor_copy(
                    v_all[:, :, g * P + HD : g * P + HD + 1],
                    ones_sb[:].rearrange("p (a b) -> p a b", b=1),
                )
                nc.gpsimd.memset(v_all[:, :, g * P + HD + 1 : (g + 1) * P], 0.0)
            # normalized attention output, feature-major per pair [128, L]
            o_sb = [qkro.tile([P, L], BF16, name=f"o_sb{p}") for p in range(N_PAIRS)]

            # ---------------- projections ----------------
            xpool = ctx.enter_context(tc.tile_pool(name="xpool", bufs=13))
            rope_t = ctx.enter_context(tc.tile_pool(name="rope", bufs=2))
            expp = ctx.enter_context(tc.tile_pool(name="expp", bufs=2 if debug else 3))
            nrm = ctx.enter_context(tc.tile_pool(name="nrm", bufs=2))
            outst = ctx.enter_context(tc.tile_pool(name="outst", bufs=2))

            def rope_chunk(dst, ps_tile, t, on_dve=False):
                """psum [128, TC] -> dst[:, t*TC:(t+1)*TC] with RoPE applied."""
                raw = rope_t.tile([P, TC], F32, name="raw")
                shuf = rope_t.tile([P, TC], F32, name="shuf")
                t1 = rope_t.tile([P, TC], F32, name="t1")
                t2 = rope_t.tile([P, TC], F32, name="t2")
                nc.vector.tensor_copy(raw[:], ps_tile[:])
                # swap 32-partition halves within each 64-row head block
                for a, b in ((0, 32), (32, 0), (64, 96), (96, 64)):
                    nc.sync.dma_start(shuf[a : a + 32, :], raw[b : b + 32, :])
                cs = cos_sb[:, ts(t, TC)]
                sn = sin_sb[:, ts(t, TC)]
                nc.vector.tensor_mul(t1[:], raw[:], cs)
                if on_dve:
                    nc.vector.tensor_mul(t2[:], shuf[:], sn)
                else:
                    nc.gpsimd.tensor_mul(t2[:], shuf[:], sn)
                nc.vector.tensor_add(dst[:, ts(t, TC)], t1[:], t2[:])

            def fetch_x(t, dma_eng):
                engs = dma_eng if isinstance(dma_eng, list) else [dma_eng]
                xts = []
                for h in range(HC):
                    xt = xpool.tile([P, TC], BF16, name="xt")
                    engs[h % len(engs)].dma_start(xt[:], xT_d[ts(h, P), ts(t, TC)])
                    xts.append(xt)
                return xts

            def v_chunk(t, xts, ps_qk):
                for s in range(TC // P):  # 4 token tiles per chunk
                    tt = t * (TC // P) + s
                    vp = ps_qk.tile(
                        [P, E_LOCAL], F32, name="vp",
                        tag=("qp" if s % 2 == 0 else "kp"), bufs=1,
                    )
                    for h in range(HC):
                        nc.tensor.matmul(
                            vp[:], xts[h][:, ts(s, P)], wv_sb[:, h, :],
                            start=(h == 0), stop=(h == HC - 1),
                        )
                    for pr in range(N_PAIRS):
                        vsrc = vp[:, ts(pr, P)].rearrange("p (g c) -> p g c", g=2)
                        dst = v4[:, tt, 2 * pr : 2 * pr + 2, 0:HD]
                        nc.vector.tensor_copy(dst, vsrc)

            def qk_proj(pair, ps_qk, with_v=False):
                for t in range(N_TC):
                    xts = fetch_x(t, nc.sync)
                    if pair == 0 and t == 0:
                        nc.gpsimd.dma_start(cos_sb[:], cos_d[:])
                        nc.gpsimd.dma_start(sin_sb[:], sin_d[:])
                    qp = ps_qk.tile([P, TC], F32, name="qp", tag="qp", bufs=1)
                    for h in range(HC):
                        nc.tensor.matmul(
                            qp[:], wq_sb[:, h, ts(pair, P)], xts[h][:],
                            start=(h == 0), stop=(h == HC - 1),
                        )
                    rope_chunk(q_ro[pair], qp, t, on_dve=(pair == 1))
                    kp = ps_qk.tile([P, TC], F32, name="kp", tag="kp", bufs=1)
                    for h in range(HC):
                        nc.tensor.matmul(
                            kp[:], wk_sb[:, h, ts(pair, P)], xts[h][:],
                            start=(h == 0), stop=(h == HC - 1),
                        )
                    rope_chunk(k_ro[pair], kp, t, on_dve=(pair == 1))
                    if with_v:
                        v_chunk(t, xts, ps_qk)


            def attention_pair(pair, ps_st, ps_ot, only_c=None):
                for c in ([only_c] if only_c is not None else range(N_TC)):
                    ot = ps_ot.tile([P, 2 * TC], F32, name="ot", bufs=1)
                    for kt in range(N_TT):
                        st = ps_st.tile([P, 2 * TC], F32, name="st", tag="st")
                        nc.tensor.matmul(
                            st[:, 0:TC],
                            k_ro[pair][0:HD, ts(kt, P)],
                            q_ro[pair][0:HD, ts(c, TC)],
                            start=True, stop=True,
                        )
                        nc.tensor.matmul(
                            st[:, TC : 2 * TC],
                            k_ro[pair][HD:P, ts(kt, P)],
                            q_ro[pair][HD:P, ts(c, TC)],
                            start=True, stop=True,
                            tile_position=(64, 0),
                        )
                        ex = expp.tile([P, 2 * TC], BF16, name="ex")
                        nc.scalar.activation(ex[:], st[:], AF.Exp, scale=SCALE)
                        for hd_i in range(2):
                            g = 2 * pair + hd_i
                            nc.tensor.matmul(
                                ot[:, ts(hd_i, TC)],
                                v_all[:, kt, ts(g, P)],
                                ex[:, ts(hd_i, TC)],
                                start=(kt == 0), stop=(kt == N_TT - 1),
                            )
                    # fast-release ot: copy unnormalized o + sums to SBUF,
                    # then normalize from SBUF so the psum bank frees early
                    oun = nrm.tile([HD + 1, 2 * TC], F32, name="oun")
                    nc.vector.tensor_copy(oun[:], ot[0 : HD + 1, :])
                    s32 = nrm.tile([32, 2 * TC // 32], F32, name="s32")
                    rp_eng = nc.sync if (pair == 1 and c == N_TC - 1) else nc.gpsimd
                    rp_eng.dma_start(
                        s32[:], oun[HD : HD + 1, :].rearrange("p (a b) -> p a b", a=32)
                    )
                    nc.vector.reciprocal(s32[:], s32[:])
                    invrow = nrm.tile([1, 2 * TC], F32, name="invrow")
                    rp_eng.dma_start(
                        invrow[:].rearrange("p (a b) -> p a b", a=32), s32[:]
                    )
                    bsum = nrm.tile([HD, 2 * TC], F32, name="bsum")
                    nc.gpsimd.partition_broadcast(bsum[:], invrow[:])
                    if debug and pair == 0 and c == 0:
                        nc.sync.dma_start(dbg_ot[0 : HD, :], oun[0:HD, :])
                        nc.sync.dma_start(dbg_ot[HD : HD + 1, :], oun[HD : HD + 1, :])
                        nc.sync.dma_start(dbg_inv[:], invrow[:])
                        nc.sync.dma_start(dbg_bsum[:], bsum[:])
                    for hd_i in range(2):
                        dsts = o_sb[pair]
                        if hd_i == 0:
                            nc.vector.tensor_mul(
                                dsts[0:HD, ts(c, TC)],
                                oun[0:HD, ts(hd_i, TC)],
                                bsum[:, ts(hd_i, TC)],
                            )
                        else:
                            onrm = nrm.tile([HD, TC], BF16, name="onrm")
                            nc.vector.tensor_mul(
                                onrm[:], oun[0:HD, ts(hd_i, TC)], bsum[:, ts(hd_i, TC)]
                            )
                            nc.gpsimd.dma_start(dsts[HD:P, ts(c, TC)], onrm[:])

            def o_proj_chunk(t, ps_qk):
                for fc in range(HC):
                    op = ps_qk.tile(
                        [P, TC], F32, name="op", tag=("qp" if fc % 2 == 0 else "kp"), bufs=1
                    )
                    for pair in range(N_PAIRS):
                        nc.tensor.matmul(
                            op[:],
                            wo_sb[:, pair, ts(fc, P)],
                            o_sb[pair][:, ts(t, TC)],
                            start=(pair == 0), stop=(pair == N_PAIRS - 1),
                        )
                    ob = outst.tile([P, TC], BF16, name="ob")
                    nc.vector.tensor_copy(ob[:], op[:])
                    nc.sync.dma_start(out_d[ts(fc, P), ts(t, TC)], ob[:])

            # ---- emission order drives scheduler priority ----
            # PSUM banks: qk{2} -> released -> oproj{2}; v{2} -> released ->
            # part of st{4}; ot{2}. Peak = 8 banks.
            with tc.tile_pool(name="ps_qk", bufs=1, space="PSUM") as ps_qk:
                with tc.tile_pool(name="ps_st", bufs=2, space="PSUM") as ps_st:
                    with tc.tile_pool(name="ps_ot", bufs=1, space="PSUM") as ps_ot:
                        nc.gpsimd.dma_start(wv_sb[:], wv_d[:].rearrange("(c p) e -> p c e", p=P))
                        qk_proj(0, ps_qk, with_v=True)
                        nc.sync.dma_start(wo_sb[:], wo_d[:].rearrange("(c p) f -> p c f", p=P))
                        attention_pair(0, ps_st, ps_ot)
                        qk_proj(1, ps_qk)
                        for c in range(N_TC):
                            attention_pair(1, ps_st, ps_ot, only_c=c)
                            if c >= 1:
                                o_proj_chunk(c - 1, ps_qk)
                        o_proj_chunk(N_TC - 1, ps_qk)

    nc.compile()
    return nc, names


_CACHE = {}


def _get_program():
    if "prog" not in _CACHE:
        _CACHE["prog"] = build_program()
    return _CACHE["prog"]


def _rope_tables():
    inv_freq = 1.0 / (BASE ** (np.arange(0, HD, 2, dtype=np.float64) / HD))
    t = np.arange(L, dtype=np.float64)
    freqs = np.outer(t, inv_freq)            # [L, 32]
    emb = np.concatenate((freqs, freqs), -1)  # [L, 64]
    cos = np.cos(emb).T.astype(np.float32)    # [64, L]
    sin = np.sin(emb).T.astype(np.float32)    # [64, L]
    sin_signed = sin.copy()
    sin_signed[: HD // 2] *= -1.0             # rotate_half sign baked in
    cosT = np.ascontiguousarray(np.concatenate([cos, cos], 0))      # [128, L]
    sinT = np.ascontiguousarray(np.concatenate([sin_signed, sin_signed], 0))
    return cosT, sinT


def make_in_maps(names, x, Wq, Wk, Wv, Wo):
    cosT, sinT = _rope_tables()
    in_maps = []
    bf = ml_dtypes.bfloat16
    xTs = [np.ascontiguousarray(x[b].T.astype(bf)) for b in range(B)]
    for core in range(8):
        b = core // 4
        g = core % 4
        es = slice(g * E_LOCAL, (g + 1) * E_LOCAL)
        m = {
            names["in"][0]: xTs[b],
            names["in"][1]: np.ascontiguousarray(Wq[es, :].T.astype(bf)),
            names["in"][2]: np.ascontiguousarray(Wk[es, :].T.astype(bf)),
            names["in"][3]: np.ascontiguousarray(Wv[es, :].T.astype(bf)),
            names["in"][4]: np.ascontiguousarray(Wo[:, es].T.astype(bf)),
            names["in"][5]: cosT,
            names["in"][6]: sinT,
        }
        in_maps.append(m)
    return in_maps


def gather_out(names, res):
    out = np.zeros((B, L, HIDDEN), dtype=np.float32)
    for b in range(B):
        acc = np.zeros((HIDDEN, L), dtype=np.float32)
        for g in range(4):
            acc += np.asarray(res.results[b * 4 + g][names["out"]]).astype(np.float32)
        out[b] = acc.T
    return out


def kernel(x, Wq, Wk, Wv, Wo):
    x = np.asarray(x, dtype=np.float32)
    Wq = np.asarray(Wq, dtype=np.float32)
    Wk = np.asarray(Wk, dtype=np.float32)
    Wv = np.asarray(Wv, dtype=np.float32)
    Wo = np.asarray(Wo, dtype=np.float32)

    nc, names = _get_program()
    in_maps = make_in_maps(names, x, Wq, Wk, Wv, Wo)
    res = run_bass_kernel_spmd(nc, in_maps, core_ids=list(range(8)))
    return gather_out(names, res)



# revision 20
# speedup vs baseline: 1.0702x; 1.0702x over previous
"""Trainium2 Bass kernel: 16-head RoPE attention (B=2, L=2048, HIDDEN=1024).

Sharding: 8 cores = 2 batches x 4 head-groups (4 heads per core).
Attention storage (rope'd q/k, v, exp output) is bf16: score/PV matmuls
issue ~24% faster than fp32r on TRN2 and halve the SBUF traffic, at
rel err ~4e-3 (vs 5e-4 all-fp32r), well inside the 2e-2 gate.
Each core computes q/k/v projections for its 4 heads (feature-major),
RoPE, scores-transposed [k,q] per head, exp (no max subtraction --
scores are ~N(0,1)), PV with a ones-column in V to get softmax sums,
normalization, and a partial output projection [1024, 2048].
Host sums the 4 partials per batch and transposes back.
"""

import numpy as np
import ml_dtypes
from contextlib import ExitStack

from concourse import bacc, tile, mybir
from concourse.bass import ts
from concourse.bass_utils import run_bass_kernel_spmd

HIDDEN = 1024
HEADS = 16
HD = 64
L = 2048
B = 2
BASE = 10000.0

P = 128
E_LOCAL = 256          # 4 heads per core
N_PAIRS = 2            # head pairs per core (2 heads on 128 partitions)
HC = HIDDEN // P       # 8 hidden chunks
TC = 512               # token chunk (matmul free dim)
N_TC = L // TC         # 4
N_TT = L // P          # 16 token tiles (for v / k-tiles)
SCALE = 1.0 / 8.0      # 1/sqrt(HD)

F32 = mybir.dt.float32
F32R = mybir.dt.float32r
BF16 = mybir.dt.bfloat16
AF = mybir.ActivationFunctionType
ALU = mybir.AluOpType


def r(ap):
    """View an fp32 AP as float32r for full-rate PE matmuls."""
    return ap.bitcast(F32R)


def build_program(debug=False):
    nc = bacc.Bacc(None, target_bir_lowering=False)
    names = {}
    with tile.TileContext(nc) as tc:
        ctx = ExitStack()
        with ctx:
            dram = ctx.enter_context(tc.tile_pool(name="dram", bufs=1, space="DRAM"))
            xT_d = dram.tile([HIDDEN, L], BF16, kind="ExternalInput", name="xT")
            wq_d = dram.tile([HIDDEN, E_LOCAL], BF16, kind="ExternalInput", name="wq")
            wk_d = dram.tile([HIDDEN, E_LOCAL], BF16, kind="ExternalInput", name="wk")
            wv_d = dram.tile([HIDDEN, E_LOCAL], BF16, kind="ExternalInput", name="wv")
            wo_d = dram.tile([E_LOCAL, HIDDEN], BF16, kind="ExternalInput", name="wo")
            cos_d = dram.tile([P, L], F32, kind="ExternalInput", name="cosT")
            sin_d = dram.tile([P, L], F32, kind="ExternalInput", name="sinT")
            out_d = dram.tile([HIDDEN, L], BF16, kind="ExternalOutput", name="outT")
            if debug:
                dbg_q = dram.tile([P, L], F32, kind="ExternalOutput", name="dbg_q")
                dbg_k = dram.tile([P, L], F32, kind="ExternalOutput", name="dbg_k")
                dbg_v = dram.tile([P, N_TT * 4 * (HD + 1)], F32, kind="ExternalOutput", name="dbg_v")
                dbg_o = dram.tile([P, L], F32, kind="ExternalOutput", name="dbg_o")
                dbg_ot = dram.tile([HD + 1, 2 * TC], F32, kind="ExternalOutput", name="dbg_ot")
                dbg_inv = dram.tile([1, 2 * TC], F32, kind="ExternalOutput", name="dbg_inv")
                dbg_bsum = dram.tile([HD, 2 * TC], F32, kind="ExternalOutput", name="dbg_bsum")
                names["dbg"] = [t.tensor.name for t in (dbg_q, dbg_k, dbg_v, dbg_o, dbg_ot, dbg_inv, dbg_bsum)]
            names["in"] = ["xT", "wq", "wk", "wv", "wo", "cosT", "sinT"]
            names["out"] = "outT"
            names["in"] = [t.tensor.name for t in (xT_d, wq_d, wk_d, wv_d, wo_d, cos_d, sin_d)]
            names["out"] = out_d.tensor.name

            # ---------------- persistent SBUF ----------------
            const = ctx.enter_context(tc.tile_pool(name="const", bufs=1))
            wq_sb = const.tile([P, HC, E_LOCAL], BF16)
            wk_sb = const.tile([P, HC, E_LOCAL], BF16)
            wv_sb = const.tile([P, HC, E_LOCAL], BF16)
            wo_sb = const.tile([P, 2, HIDDEN], BF16)
            cos_sb = const.tile([P, L], F32)
            sin_sb = const.tile([P, L], F32)
            nc.sync.dma_start(wq_sb[:], wq_d[:].rearrange("(c p) e -> p c e", p=P))
            nc.gpsimd.dma_start(wk_sb[:], wk_d[:].rearrange("(c p) e -> p c e", p=P))

            # rope'd q and k, feature-major: per pair [128, L]
            qkro = ctx.enter_context(tc.tile_pool(name="qkro", bufs=1))
            q_ro = [qkro.tile([P, L], BF16, name=f"q_ro{p}") for p in range(N_PAIRS)]
            k_ro = [qkro.tile([P, L], BF16, name=f"k_ro{p}") for p in range(N_PAIRS)]
            # v token-major with ones columns: [128 tok, tt, 4*65]
            v_all = qkro.tile([P, N_TT, 4 * P], BF16)
            v4 = v_all[:].rearrange("p t (g c) -> p t g c", g=4)
            ones_sb = qkro.tile([P, N_TT], F32)
            nc.vector.memset(ones_sb[:], 1.0)
            for g in range(4):
                nc.vector.tensor_copy(
                    v_all[:, :, g * P + HD : g * P + HD + 1],
                    ones_sb[:].rearrange("p (a b) -> p a b", b=1),
                )
                nc.gpsimd.memset(v_all[:, :, g * P + HD + 1 : (g + 1) * P], 0.0)
            # normalized attention output, feature-major per pair [128, L]
            o_sb = [qkro.tile([P, L], BF16, name=f"o_sb{p}") for p in range(N_PAIRS)]

            # ---------------- projections ----------------
            xpool = ctx.enter_context(tc.tile_pool(name="xpool", bufs=13))
            rope_t = ctx.enter_context(tc.tile_pool(name="rope", bufs=2))
            expp = ctx.enter_context(tc.tile_pool(name="expp", bufs=2 if debug else 3))
            nrm = ctx.enter_context(tc.tile_pool(name="nrm", bufs=2))
            outst = ctx.enter_context(tc.tile_pool(name="outst", bufs=2))

            def rope_chunk(dst, ps_tile, t, on_dve=False):
                """psum [128, TC] -> dst[:, t*TC:(t+1)*TC] with RoPE applied."""
                raw = rope_t.tile([P, TC], F32, name="raw")
                shuf = rope_t.tile([P, TC], F32, name="shuf")
                t1 = rope_t.tile([P, TC], F32, name="t1")
                t2 = rope_t.tile([P, TC], F32, name="t2")
                nc.vector.tensor_copy(raw[:], ps_tile[:])
                # swap 32-partition halves within each 64-row head block
                for a, b in ((0, 32), (32, 0), (64, 96), (96, 64)):
                    nc.sync.dma_start(shuf[a : a + 32, :], raw[b : b + 32, :])
                cs = cos_sb[:, ts(t, TC)]
                sn = sin_sb[:, ts(t, TC)]
                nc.vector.tensor_mul(t1[:], raw[:], cs)
                if on_dve:
                    nc.vector.tensor_mul(t2[:], shuf[:], sn)
                else:
                    nc.gpsimd.tensor_mul(t2[:], shuf[:], sn)
                nc.vector.tensor_add(dst[:, ts(t, TC)], t1[:], t2[:])

            def fetch_x(t, dma_eng):
                engs = dma_eng if isinstance(dma_eng, list) else [dma_eng]
                xts = []
                for h in range(HC):
                    xt = xpool.tile([P, TC], BF16, name="xt")
                    engs[h % len(engs)].dma_start(xt[:], xT_d[ts(h, P), ts(t, TC)])
                    xts.append(xt)
                return xts

            def v_chunk(t, xts, ps_qk):
                for s in range(TC // P):  # 4 token tiles per chunk
                    tt = t * (TC // P) + s
                    vp = ps_qk.tile(
                        [P, E_LOCAL], F32, name="vp",
                        tag=("qp" if s % 2 == 0 else "kp"), bufs=1,
                    )
                    for h in range(HC):
                        nc.tensor.matmul(
                            vp[:], xts[h][:, ts(s, P)], wv_sb[:, h, :],
                            start=(h == 0), stop=(h == HC - 1),
                        )
                    for pr in range(N_PAIRS):
                        vsrc = vp[:, ts(pr, P)].rearrange("p (g c) -> p g c", g=2)
                        dst = v4[:, tt, 2 * pr : 2 * pr + 2, 0:HD]
                        nc.vector.tensor_copy(dst, vsrc)

            def qk_proj(pair, ps_qk, with_v=False):
                for t in range(N_TC):
                    xts = fetch_x(t, nc.sync)
                    if pair == 0 and t == 0:
                        nc.gpsimd.dma_start(cos_sb[:], cos_d[:])
                        nc.gpsimd.dma_start(sin_sb[:], sin_d[:])
                    qp = ps_qk.tile([P, TC], F32, name="qp", tag="qp", bufs=1)
                    for h in range(HC):
                        nc.tensor.matmul(
                            qp[:], wq_sb[:, h, ts(pair, P)], xts[h][:],
                            start=(h == 0), stop=(h == HC - 1),
                        )
                    rope_chunk(q_ro[pair], qp, t, on_dve=(pair == 1))
                    kp = ps_qk.tile([P, TC], F32, name="kp", tag="kp", bufs=1)
                    for h in range(HC):
                        nc.tensor.matmul(
                            kp[:], wk_sb[:, h, ts(pair, P)], xts[h][:],
                            start=(h == 0), stop=(h == HC - 1),
                        )
                    rope_chunk(k_ro[pair], kp, t, on_dve=(pair == 1))
                    if with_v:
                        v_chunk(t, xts, ps_qk)


            def attention_pair(pair, ps_st, ps_ot, only_c=None):
                for c in ([only_c] if only_c is not None else range(N_TC)):
                    ot = ps_ot.tile([P, 2 * TC], F32, name="ot", bufs=1)
                    for kt in range(N_TT):
                        st = ps_st.tile([P, 2 * TC], F32, name="st", tag="st")
                        nc.tensor.matmul(
                            st[:, 0:TC],
                            k_ro[pair][0:HD, ts(kt, P)],
                            q_ro[pair][0:HD, ts(c, TC)],
                            start=True, stop=True,
                        )
                        nc.tensor.matmul(
                            st[:, TC : 2 * TC],
                            k_ro[pair][HD:P, ts(kt, P)],
                            q_ro[pair][HD:P, ts(c, TC)],
                            start=True, stop=True,
                            tile_position=(64, 0),
                        )
                        ex = expp.tile([P, 2 * TC], BF16, name="ex")
                        nc.scalar.activation(ex[:], st[:], AF.Exp, scale=SCALE)
                        for hd_i in range(2):
                            g = 2 * pair + hd_i
                            nc.tensor.matmul(
                                ot[:, ts(hd_i, TC)],
                                v_all[:, kt, ts(g, P)],
                                ex[:, ts(hd_i, TC)],
                                start=(kt == 0), stop=(kt == N_TT - 1),
                            )
                    # fast-release ot: copy unnormalized o + sums to SBUF,
                    # then normalize from SBUF so the psum bank frees early
                    oun = nrm.tile([HD + 1, 2 * TC], F32, name="oun")
                    nc.vector.tensor_copy(oun[:], ot[0 : HD + 1, :])
                    s32 = nrm.tile([32, 2 * TC // 32], F32, name="s32")
                    rp_eng = nc.sync if (pair == 1 and c == N_TC - 1) else nc.gpsimd
                    rp_eng.dma_start(
                        s32[:], oun[HD : HD + 1, :].rearrange("p (a b) -> p a b", a=32)
                    )
                    nc.vector.reciprocal(s32[:], s32[:])
                    invrow = nrm.tile([1, 2 * TC], F32, name="invrow")
                    rp_eng.dma_start(
                        invrow[:].rearrange("p (a b) -> p a b", a=32), s32[:]
                    )
                    bsum = nrm.tile([HD, 2 * TC], F32, name="bsum")
                    nc.gpsimd.partition_broadcast(bsum[:], invrow[:])
                    if debug and pair == 0 and c == 0:
                        nc.sync.dma_start(dbg_ot[0 : HD, :], oun[0:HD, :])
                        nc.sync.dma_start(dbg_ot[HD : HD + 1, :], oun[HD : HD + 1, :])
                        nc.sync.dma_start(dbg_inv[:], invrow[:])
                        nc.sync.dma_start(dbg_bsum[:], bsum[:])
                    for hd_i in range(2):
                        dsts = o_sb[pair]
                        if hd_i == 0:
                            nc.vector.tensor_mul(
                                dsts[0:HD, ts(c, TC)],
                                oun[0:HD, ts(hd_i, TC)],
                                bsum[:, ts(hd_i, TC)],
                            )
                        else:
                            onrm = nrm.tile([HD, TC], BF16, name="onrm")
                            nc.vector.tensor_mul(
                                onrm[:], oun[0:HD, ts(hd_i, TC)], bsum[:, ts(hd_i, TC)]
                            )
                            nc.gpsimd.dma_start(dsts[HD:P, ts(c, TC)], onrm[:])

            def o_proj_chunk(t, ps_qk):
                for fc in range(HC):
                    op = ps_qk.tile(
                        [P, TC], F32, name="op", tag=("qp" if fc % 2 == 0 else "kp"), bufs=1
                    )
                    for pair in range(N_PAIRS):
                        nc.tensor.matmul(
                            op[:],
                            wo_sb[:, pair, ts(fc, P)],
                            o_sb[pair][:, ts(t, TC)],
                            start=(pair == 0), stop=(pair == N_PAIRS - 1),
                        )
                    ob = outst.tile([P, TC], BF16, name="ob")
                    nc.vector.tensor_copy(ob[:], op[:])
                    nc.sync.dma_start(out_d[ts(fc, P), ts(t, TC)], ob[:])

            # ---- emission order drives scheduler priority ----
            # PSUM banks: qk{2} -> released -> oproj{2}; v{2} -> released ->
            # part of st{4}; ot{2}. Peak = 8 banks.
            with tc.tile_pool(name="ps_qk", bufs=1, space="PSUM") as ps_qk:
                with tc.tile_pool(name="ps_st", bufs=2, space="PSUM") as ps_st:
                    with tc.tile_pool(name="ps_ot", bufs=1, space="PSUM") as ps_ot:
                        nc.gpsimd.dma_start(wv_sb[:], wv_d[:].rearrange("(c p) e -> p c e", p=P))
                        qk_proj(0, ps_qk, with_v=True)
                        nc.sync.dma_start(wo_sb[:], wo_d[:].rearrange("(c p) f -> p c f", p=P))
                        # interleave the two pairs per chunk so o_proj and
                        # normalize spread across the whole attention span
                        attention_pair(0, ps_st, ps_ot, only_c=0)
                        qk_proj(1, ps_qk)
                        for c in range(N_TC):
                            if c >= 1:
                                attention_pair(0, ps_st, ps_ot, only_c=c)
                            attention_pair(1, ps_st, ps_ot, only_c=c)
                            if c >= 1:
                                o_proj_chunk(c - 1, ps_qk)
                        o_proj_chunk(N_TC - 1, ps_qk)

    nc.compile()
    return nc, names


_CACHE = {}


def _get_program():
    if "prog" not in _CACHE:
        _CACHE["prog"] = build_program()
    return _CACHE["prog"]


def _rope_tables():
    inv_freq = 1.0 / (BASE ** (np.arange(0, HD, 2, dtype=np.float64) / HD))
    t = np.arange(L, dtype=np.float64)
    freqs = np.outer(t, inv_freq)            # [L, 32]
    emb = np.concatenate((freqs, freqs), -1)  # [L, 64]
    cos = np.cos(emb).T.astype(np.float32)    # [64, L]
    sin = np.sin(emb).T.astype(np.float32)    # [64, L]
    sin_signed = sin.copy()
    sin_signed[: HD // 2] *= -1.0             # rotate_half sign baked in
    cosT = np.ascontiguousarray(np.concatenate([cos, cos], 0))      # [128, L]
    sinT = np.ascontiguousarray(np.concatenate([sin_signed, sin_signed], 0))
    return cosT, sinT


def make_in_maps(names, x, Wq, Wk, Wv, Wo):
    cosT, sinT = _rope_tables()
    in_maps = []
    bf = ml_dtypes.bfloat16
    xTs = [np.ascontiguousarray(x[b].T.astype(bf)) for b in range(B)]
    for core in range(8):
        b = core // 4
        g = core % 4
        es = slice(g * E_LOCAL, (g + 1) * E_LOCAL)
        m = {
            names["in"][0]: xTs[b],
            names["in"][1]: np.ascontiguousarray(Wq[es, :].T.astype(bf)),
            names["in"][2]: np.ascontiguousarray(Wk[es, :].T.astype(bf)),
            names["in"][3]: np.ascontiguousarray(Wv[es, :].T.astype(bf)),
            names["in"][4]: np.ascontiguousarray(Wo[:, es].T.astype(bf)),
            names["in"][5]: cosT,
            names["in"][6]: sinT,
        }
        in_maps.append(m)
    return in_maps


def gather_out(names, res):
    out = np.zeros((B, L, HIDDEN), dtype=np.float32)
    for b in range(B):
        acc = np.zeros((HIDDEN, L), dtype=np.float32)
        for g in range(4):
            acc += np.asarray(res.results[b * 4 + g][names["out"]]).astype(np.float32)
        out[b] = acc.T
    return out


def kernel(x, Wq, Wk, Wv, Wo):
    x = np.asarray(x, dtype=np.float32)
    Wq = np.asarray(Wq, dtype=np.float32)
    Wk = np.asarray(Wk, dtype=np.float32)
    Wv = np.asarray(Wv, dtype=np.float32)
    Wo = np.asarray(Wo, dtype=np.float32)

    nc, names = _get_program()
    in_maps = make_in_maps(names, x, Wq, Wk, Wv, Wo)
    res = run_bass_kernel_spmd(nc, in_maps, core_ids=list(range(8)))
    return gather_out(names, res)



# revision 21
# speedup vs baseline: 1.1642x; 1.0879x over previous
"""Trainium2 Bass kernel: 16-head RoPE attention (B=2, L=2048, HIDDEN=1024).

Sharding: 8 cores = 2 batches x 4 head-groups (4 heads per core).
Attention storage (rope'd q/k, v, exp output) is bf16: score/PV matmuls
issue ~24% faster than fp32r on TRN2 and halve the SBUF traffic, at
rel err ~4e-3 (vs 5e-4 all-fp32r), well inside the 2e-2 gate.
Each core computes q/k/v projections for its 4 heads (feature-major),
RoPE, scores-transposed [k,q] per head, exp (no max subtraction --
scores are ~N(0,1)), PV with a ones-column in V to get softmax sums,
normalization, and a partial output projection [1024, 2048].
Host sums the 4 partials per batch and transposes back.
"""

import numpy as np
import ml_dtypes
from contextlib import ExitStack

from concourse import bacc, tile, mybir
from concourse.bass import ts
from concourse.bass_utils import run_bass_kernel_spmd

HIDDEN = 1024
HEADS = 16
HD = 64
L = 2048
B = 2
BASE = 10000.0

P = 128
E_LOCAL = 256          # 4 heads per core
N_PAIRS = 2            # head pairs per core (2 heads on 128 partitions)
HC = HIDDEN // P       # 8 hidden chunks
TC = 512               # token chunk (matmul free dim)
N_TC = L // TC         # 4
N_TT = L // P          # 16 token tiles (for v / k-tiles)
SCALE = 1.0 / 8.0      # 1/sqrt(HD)

F32 = mybir.dt.float32
F32R = mybir.dt.float32r
BF16 = mybir.dt.bfloat16
AF = mybir.ActivationFunctionType
ALU = mybir.AluOpType


def r(ap):
    """View an fp32 AP as float32r for full-rate PE matmuls."""
    return ap.bitcast(F32R)


def build_program(debug=False):
    nc = bacc.Bacc(None, target_bir_lowering=False)
    names = {}
    with tile.TileContext(nc) as tc:
        ctx = ExitStack()
        with ctx:
            dram = ctx.enter_context(tc.tile_pool(name="dram", bufs=1, space="DRAM"))
            xT_d = dram.tile([HIDDEN, L], BF16, kind="ExternalInput", name="xT")
            wq_d = dram.tile([HIDDEN, E_LOCAL], BF16, kind="ExternalInput", name="wq")
            wk_d = dram.tile([HIDDEN, E_LOCAL], BF16, kind="ExternalInput", name="wk")
            wv_d = dram.tile([HIDDEN, E_LOCAL], BF16, kind="ExternalInput", name="wv")
            wo_d = dram.tile([E_LOCAL, HIDDEN], BF16, kind="ExternalInput", name="wo")
            cos_d = dram.tile([P, L], F32, kind="ExternalInput", name="cosT")
            sin_d = dram.tile([P, L], F32, kind="ExternalInput", name="sinT")
            out_d = dram.tile([HIDDEN, L], BF16, kind="ExternalOutput", name="outT")
            if debug:
                dbg_q = dram.tile([P, L], F32, kind="ExternalOutput", name="dbg_q")
                dbg_k = dram.tile([P, L], F32, kind="ExternalOutput", name="dbg_k")
                dbg_v = dram.tile([P, N_TT * 4 * (HD + 1)], F32, kind="ExternalOutput", name="dbg_v")
                dbg_o = dram.tile([P, L], F32, kind="ExternalOutput", name="dbg_o")
                dbg_ot = dram.tile([HD + 1, 2 * TC], F32, kind="ExternalOutput", name="dbg_ot")
                dbg_inv = dram.tile([1, 2 * TC], F32, kind="ExternalOutput", name="dbg_inv")
                dbg_bsum = dram.tile([HD, 2 * TC], F32, kind="ExternalOutput", name="dbg_bsum")
                names["dbg"] = [t.tensor.name for t in (dbg_q, dbg_k, dbg_v, dbg_o, dbg_ot, dbg_inv, dbg_bsum)]
            names["in"] = ["xT", "wq", "wk", "wv", "wo", "cosT", "sinT"]
            names["out"] = "outT"
            names["in"] = [t.tensor.name for t in (xT_d, wq_d, wk_d, wv_d, wo_d, cos_d, sin_d)]
            names["out"] = out_d.tensor.name

            # ---------------- persistent SBUF ----------------
            const = ctx.enter_context(tc.tile_pool(name="const", bufs=1))
            wq_sb = const.tile([P, HC, E_LOCAL], BF16)
            wk_sb = const.tile([P, HC, E_LOCAL], BF16)
            wv_sb = const.tile([P, HC, E_LOCAL], BF16)
            wo_sb = const.tile([P, 2, HIDDEN], BF16)
            cos_sb = const.tile([P, L], F32)
            sin_sb = const.tile([P, L], F32)
            nc.sync.dma_start(wq_sb[:], wq_d[:].rearrange("(c p) e -> p c e", p=P))
            nc.gpsimd.dma_start(wk_sb[:], wk_d[:].rearrange("(c p) e -> p c e", p=P))

            # rope'd q and k, feature-major: per pair [128, L]
            qkro = ctx.enter_context(tc.tile_pool(name="qkro", bufs=1))
            q_ro = [qkro.tile([P, L], BF16, name=f"q_ro{p}") for p in range(N_PAIRS)]
            k_ro = [qkro.tile([P, L], BF16, name=f"k_ro{p}") for p in range(N_PAIRS)]
            # v token-major with ones columns: [128 tok, tt, 4*65]
            v_all = qkro.tile([P, N_TT, 4 * P], BF16)
            v4 = v_all[:].rearrange("p t (g c) -> p t g c", g=4)
            ones_sb = qkro.tile([P, N_TT], F32)
            nc.vector.memset(ones_sb[:], 1.0)
            for g in range(4):
                nc.vector.tensor_copy(
                    v_all[:, :, g * P + HD : g * P + HD + 1],
                    ones_sb[:].rearrange("p (a b) -> p a b", b=1),
                )
                nc.gpsimd.memset(v_all[:, :, g * P + HD + 1 : (g + 1) * P], 0.0)
            # normalized attention output, feature-major per pair [128, L]
            o_sb = [qkro.tile([P, L], BF16, name=f"o_sb{p}") for p in range(N_PAIRS)]

            # ---------------- projections ----------------
            xpool = ctx.enter_context(tc.tile_pool(name="xpool", bufs=13))
            rope_t = ctx.enter_context(tc.tile_pool(name="rope", bufs=2))
            expp = ctx.enter_context(tc.tile_pool(name="expp", bufs=2 if debug else 3))
            nrm = ctx.enter_context(tc.tile_pool(name="nrm", bufs=2))
            outst = ctx.enter_context(tc.tile_pool(name="outst", bufs=2))

            def rope_chunk(dst, ps_tile, t, on_dve=False):
                """psum [128, TC] -> dst[:, t*TC:(t+1)*TC] with RoPE applied."""
                raw = rope_t.tile([P, TC], F32, name="raw")
                shuf = rope_t.tile([P, TC], F32, name="shuf")
                t1 = rope_t.tile([P, TC], F32, name="t1")
                t2 = rope_t.tile([P, TC], F32, name="t2")
                nc.vector.tensor_copy(raw[:], ps_tile[:])
                # swap 32-partition halves within each 64-row head block
                for a, b in ((0, 32), (32, 0), (64, 96), (96, 64)):
                    nc.sync.dma_start(shuf[a : a + 32, :], raw[b : b + 32, :])
                cs = cos_sb[:, ts(t, TC)]
                sn = sin_sb[:, ts(t, TC)]
                nc.vector.tensor_mul(t1[:], raw[:], cs)
                if on_dve:
                    nc.vector.tensor_mul(t2[:], shuf[:], sn)
                else:
                    nc.gpsimd.tensor_mul(t2[:], shuf[:], sn)
                nc.vector.tensor_add(dst[:, ts(t, TC)], t1[:], t2[:])

            def fetch_x(t, dma_eng):
                engs = dma_eng if isinstance(dma_eng, list) else [dma_eng]
                xts = []
                for h in range(HC):
                    xt = xpool.tile([P, TC], BF16, name="xt")
                    engs[h % len(engs)].dma_start(xt[:], xT_d[ts(h, P), ts(t, TC)])
                    xts.append(xt)
                return xts

            def v_chunk(t, xts, ps_qk):
                for s in range(TC // P):  # 4 token tiles per chunk
                    tt = t * (TC // P) + s
                    vp = ps_qk.tile(
                        [P, E_LOCAL], F32, name="vp",
                        tag=("qp" if s % 2 == 0 else "kp"), bufs=1,
                    )
                    for h in range(HC):
                        nc.tensor.matmul(
                            vp[:], xts[h][:, ts(s, P)], wv_sb[:, h, :],
                            start=(h == 0), stop=(h == HC - 1),
                        )
                    for pr in range(N_PAIRS):
                        vsrc = vp[:, ts(pr, P)].rearrange("p (g c) -> p g c", g=2)
                        dst = v4[:, tt, 2 * pr : 2 * pr + 2, 0:HD]
                        nc.vector.tensor_copy(dst, vsrc)

            def qk_proj(pair, ps_qk, with_v=False):
                for t in range(N_TC):
                    xts = fetch_x(t, nc.sync)
                    if pair == 0 and t == 0:
                        nc.gpsimd.dma_start(cos_sb[:], cos_d[:])
                        nc.gpsimd.dma_start(sin_sb[:], sin_d[:])
                    qp = ps_qk.tile([P, TC], F32, name="qp", tag="qp", bufs=1)
                    for h in range(HC):
                        nc.tensor.matmul(
                            qp[:], wq_sb[:, h, ts(pair, P)], xts[h][:],
                            start=(h == 0), stop=(h == HC - 1),
                        )
                    rope_chunk(q_ro[pair], qp, t, on_dve=(pair == 1))
                    kp = ps_qk.tile([P, TC], F32, name="kp", tag="kp", bufs=1)
                    for h in range(HC):
                        nc.tensor.matmul(
                            kp[:], wk_sb[:, h, ts(pair, P)], xts[h][:],
                            start=(h == 0), stop=(h == HC - 1),
                        )
                    rope_chunk(k_ro[pair], kp, t, on_dve=(pair == 1))
                    if with_v:
                        v_chunk(t, xts, ps_qk)


            def attention_pair(pair, ps_st, ps_ot, only_c=None):
                for c in ([only_c] if only_c is not None else range(N_TC)):
                    ot = ps_ot.tile([P, 2 * TC], F32, name="ot", bufs=1)
                    for kt in range(N_TT):
                        st = ps_st.tile([P, 2 * TC], F32, name="st", tag="st")
                        nc.tensor.matmul(
                            st[:, 0:TC],
                            k_ro[pair][0:HD, ts(kt, P)],
                            q_ro[pair][0:HD, ts(c, TC)],
                            start=True, stop=True,
                        )
                        nc.tensor.matmul(
                            st[:, TC : 2 * TC],
                            k_ro[pair][HD:P, ts(kt, P)],
                            q_ro[pair][HD:P, ts(c, TC)],
                            start=True, stop=True,
                            tile_position=(64, 0),
                        )
                        ex = expp.tile([P, 2 * TC], BF16, name="ex")
                        nc.scalar.activation(ex[:], st[:], AF.Exp, scale=SCALE)
                        for hd_i in range(2):
                            g = 2 * pair + hd_i
                            nc.tensor.matmul(
                                ot[:, ts(hd_i, TC)],
                                v_all[:, kt, ts(g, P)],
                                ex[:, ts(hd_i, TC)],
                                start=(kt == 0), stop=(kt == N_TT - 1),
                            )
                    # fast-release ot: copy unnormalized o + sums to SBUF,
                    # then normalize from SBUF so the psum bank frees early
                    oun = nrm.tile([HD + 1, 2 * TC], F32, name="oun")
                    nc.vector.tensor_copy(oun[:], ot[0 : HD + 1, :])
                    s32 = nrm.tile([32, 2 * TC // 32], F32, name="s32")
                    rp_eng = nc.sync if (pair == 1 and c == N_TC - 1) else nc.gpsimd
                    rp_eng.dma_start(
                        s32[:], oun[HD : HD + 1, :].rearrange("p (a b) -> p a b", a=32)
                    )
                    nc.vector.reciprocal(s32[:], s32[:])
                    invrow = nrm.tile([1, 2 * TC], F32, name="invrow")
                    rp_eng.dma_start(
                        invrow[:].rearrange("p (a b) -> p a b", a=32), s32[:]
                    )
                    bsum = nrm.tile([HD, 2 * TC], F32, name="bsum")
                    nc.gpsimd.partition_broadcast(bsum[:], invrow[:])
                    if debug and pair == 0 and c == 0:
                        nc.sync.dma_start(dbg_ot[0 : HD, :], oun[0:HD, :])
                        nc.sync.dma_start(dbg_ot[HD : HD + 1, :], oun[HD : HD + 1, :])
                        nc.sync.dma_start(dbg_inv[:], invrow[:])
                        nc.sync.dma_start(dbg_bsum[:], bsum[:])
                    for hd_i in range(2):
                        dsts = o_sb[pair]
                        if hd_i == 0:
                            nc.vector.tensor_mul(
                                dsts[0:HD, ts(c, TC)],
                                oun[0:HD, ts(hd_i, TC)],
                                bsum[:, ts(hd_i, TC)],
                            )
                        else:
                            onrm = nrm.tile([HD, TC], BF16, name="onrm")
                            nc.vector.tensor_mul(
                                onrm[:], oun[0:HD, ts(hd_i, TC)], bsum[:, ts(hd_i, TC)]
                            )
                            nc.gpsimd.dma_start(dsts[HD:P, ts(c, TC)], onrm[:])

            def o_proj_chunk(t, ps_qk):
                for fc in range(HC):
                    op = ps_qk.tile(
                        [P, TC], F32, name="op", tag=("qp" if fc % 2 == 0 else "kp"), bufs=1
                    )
                    for pair in range(N_PAIRS):
                        nc.tensor.matmul(
                            op[:],
                            wo_sb[:, pair, ts(fc, P)],
                            o_sb[pair][:, ts(t, TC)],
                            start=(pair == 0), stop=(pair == N_PAIRS - 1),
                        )
                    ob = outst.tile([P, TC], BF16, name="ob")
                    nc.vector.tensor_copy(ob[:], op[:])
                    nc.sync.dma_start(out_d[ts(fc, P), ts(t, TC)], ob[:])

            # ---- emission order drives scheduler priority ----
            # PSUM banks: qk{2} -> released -> oproj{2}; v{2} -> released ->
            # part of st{4}; ot{2}. Peak = 8 banks.
            with tc.tile_pool(name="ps_qk", bufs=1, space="PSUM") as ps_qk:
                with tc.tile_pool(name="ps_st", bufs=2, space="PSUM") as ps_st:
                    with tc.tile_pool(name="ps_ot", bufs=1, space="PSUM") as ps_ot:
                        nc.gpsimd.dma_start(wv_sb[:], wv_d[:].rearrange("(c p) e -> p c e", p=P))
                        qk_proj(0, ps_qk, with_v=True)
                        nc.sync.dma_start(wo_sb[:], wo_d[:].rearrange("(c p) f -> p c f", p=P))
                        attention_pair(0, ps_st, ps_ot)
                        qk_proj(1, ps_qk)
                        for c in range(N_TC):
                            attention_pair(1, ps_st, ps_ot, only_c=c)
                            if c >= 1:
                                o_proj_chunk(c - 1, ps_qk)
                        o_proj_chunk(N_TC - 1, ps_qk)

    nc.compile()
    return nc, names


_CACHE = {}


def _get_program():
    if "prog" not in _CACHE:
        _CACHE["prog"] = build_program()
    return _CACHE["prog"]


def _rope_tables():
    inv_freq = 1.0 / (BASE ** (np.arange(0, HD, 2, dtype=np.float64) / HD))
    t = np.arange(L, dtype=np.float64)
    freqs = np.outer(t, inv_freq)            # [L, 32]
    emb = np.concatenate((freqs, freqs), -1)  # [L, 64]
    cos = np.cos(emb).T.astype(np.float32)    # [64, L]
    sin = np.sin(emb).T.astype(np.float32)    # [64, L]
    sin_signed = sin.copy()
    sin_signed[: HD // 2] *= -1.0             # rotate_half sign baked in
    cosT = np.ascontiguousarray(np.concatenate([cos, cos], 0))      # [128, L]
    sinT = np.ascontiguousarray(np.concatenate([sin_signed, sin_signed], 0))
    return cosT, sinT


def make_in_maps(names, x, Wq, Wk, Wv, Wo):
    cosT, sinT = _rope_tables()
    in_maps = []
    bf = ml_dtypes.bfloat16
    xTs = [np.ascontiguousarray(x[b].T.astype(bf)) for b in range(B)]
    for core in range(8):
        b = core // 4
        g = core % 4
        es = slice(g * E_LOCAL, (g + 1) * E_LOCAL)
        m = {
            names["in"][0]: xTs[b],
            names["in"][1]: np.ascontiguousarray(Wq[es, :].T.astype(bf)),
            names["in"][2]: np.ascontiguousarray(Wk[es, :].T.astype(bf)),
            names["in"][3]: np.ascontiguousarray(Wv[es, :].T.astype(bf)),
            names["in"][4]: np.ascontiguousarray(Wo[:, es].T.astype(bf)),
            names["in"][5]: cosT,
            names["in"][6]: sinT,
        }
        in_maps.append(m)
    return in_maps


def gather_out(names, res):
    out = np.zeros((B, L, HIDDEN), dtype=np.float32)
    for b in range(B):
        acc = np.zeros((HIDDEN, L), dtype=np.float32)
        for g in range(4):
            acc += np.asarray(res.results[b * 4 + g][names["out"]]).astype(np.float32)
        out[b] = acc.T
    return out


def kernel(x, Wq, Wk, Wv, Wo):
    x = np.asarray(x, dtype=np.float32)
    Wq = np.asarray(Wq, dtype=np.float32)
    Wk = np.asarray(Wk, dtype=np.float32)
    Wv = np.asarray(Wv, dtype=np.float32)
    Wo = np.asarray(Wo, dtype=np.float32)

    nc, names = _get_program()
    in_maps = make_in_maps(names, x, Wq, Wk, Wv, Wo)
    res = run_bass_kernel_spmd(nc, in_maps, core_ids=list(range(8)))
    return gather_out(names, res)

